# revision 3
# baseline (speedup 1.0000x reference)
"""Causal self-attention with RoPE on 8 Trainium2 NeuronCores.

Problem: B=4, T=2048, C=1024, NH=16, D=64. y = proj(attn(rope(qkv(x)))).

Sharding: core = (batch b, head-group hg): 4 batches x 2 groups of 8 heads.
Each core computes its 8 heads' attention for its batch plus the partial
output projection over its 512 head-channels; the host sums the two
partials per batch and adds b_proj.

On-device layout is "transposed" throughout ([feature partitions, token
free-dim]) so no on-chip transposes are needed:
  - qT/kT produced as [d, t] directly from the QKV matmul
  - RoPE rotate_half done with a constant rotation matmul + elementwise
  - scoresT[kv, q] = kT.T-slice @ qT-slice per 128-kv tile
  - softmax denominator via a ones-column appended to V (free on PE)
  - PV gives yT[d, q]; normalization via reciprocal + partition broadcast
  - output projection consumes yT tiles directly as the stationary operand
All matmul operands are bf16 (fast weight load + no fp32 moving-width
penalty); accumulation stays f32 in PSUM. Measured rel err ~4e-3.

Scheduling: the attention phase of group g is exp-paced on the scalar
engine, so the PE work of the next group's QKV/V and the previous
group's output projection is emitted in chunks INSIDE the attention
pair loops (right after the pipeline-fill QKs) to keep the in-order PE
queue fed while exp catches up.
"""
import math
from contextlib import ExitStack

import numpy as np

import concourse.bass as bass
import concourse.tile as tile
from concourse import bacc, mybir
from concourse.bass_utils import run_bass_kernel_spmd

B, T, C, NH, D = 4, 2048, 1024, 16, 64
P = 128                 # partitions
GN = 512                # token-group size
TG = T // GN            # 4 token groups
KT = C // P             # 8 contraction tiles over C
NCORES = 8
HPC = 8                 # heads per core
f32 = mybir.dt.float32
bf16 = mybir.dt.bfloat16
AF = mybir.ActivationFunctionType
BF_NP = mybir.dt.np(bf16)

_NC_CACHE = None


def _body(ctx, tc, xT, wqkT, wvT, wpT, bqk, bvb, cosT, sinT, rmat, dmask,
          outp):
    nc = tc.nc

    const = ctx.enter_context(tc.tile_pool(name="const", bufs=1))
    resid = ctx.enter_context(tc.tile_pool(name="resid", bufs=1))
    xpool = ctx.enter_context(tc.tile_pool(name="xpool", bufs=16))
    cspool = ctx.enter_context(tc.tile_pool(name="cspool", bufs=2))
    qpool = ctx.enter_context(tc.tile_pool(name="qpool", bufs=2))
    ypool = ctx.enter_context(tc.tile_pool(name="ypool", bufs=2))
    rawp = ctx.enter_context(tc.tile_pool(name="rawp", bufs=2))
    tmpp = ctx.enter_context(tc.tile_pool(name="tmpp", bufs=2))
    attp = ctx.enter_context(tc.tile_pool(name="attp", bufs=3))
    bcp = ctx.enter_context(tc.tile_pool(name="bcp", bufs=2))
    rcp = ctx.enter_context(tc.tile_pool(name="rcp", bufs=4))
    outsb = ctx.enter_context(tc.tile_pool(name="outsb", bufs=2))
    psmm = ctx.enter_context(tc.tile_pool(name="psmm", bufs=2, space="PSUM"))
    pssc = ctx.enter_context(tc.tile_pool(name="pssc", bufs=2, space="PSUM"))
    psy = ctx.enter_context(tc.tile_pool(name="psy", bufs=2, space="PSUM"))

    # ---- constants / resident tensors ----
    wqk_t = [const.tile([P, 1024], bf16, tag=f"wqk{k}", name=f"wqk{k}")
             for k in range(KT)]
    # wv/wp loads are deferred so the startup DMA bandwidth goes to
    # x/wqk first.
    wv_t = [const.tile([P, 512], bf16, tag=f"wv{k}", name=f"wv{k}")
            for k in range(KT)]
    wp_t = [const.tile([P, 1024], bf16, tag=f"wp{k}", name=f"wp{k}")
            for k in range(4)]
    rmat_t = const.tile([P, P], bf16, tag="rmat", name="rmat_t")
    nc.sync.dma_start(rmat_t[:], rmat[:])
    dmask_t = const.tile([P, P], bf16, tag="dmask", name="dmask_t")
    nc.sync.dma_start(dmask_t[:], dmask[:])
    bqk_t = const.tile([P, 8], f32, tag="bqk", name="bqk_t")
    nc.sync.dma_start(bqk_t[:], bqk[:])
    bvb_t = const.tile([P, 512], bf16, tag="bvb", name="bvb_t")
    nc.sync.dma_start(bvb_t[:], bvb[:])

    kT_t = [resid.tile([P, T], bf16, tag=f"kT{p}", name=f"kT{p}")
            for p in range(4)]
    # vplus layout: [128 tok, tt(16) x head(8) x (64 d + 1 ones)]
    vplus = resid.tile([P, 16 * HPC * 65], bf16, tag="vplus", name="vplus")
    vp4 = vplus[:].rearrange("p (t h e) -> p t h e", t=16, h=HPC)
    nc.vector.memset(vp4[:, :, :, 64:65], 1.0)

    # ---- per-group building blocks ----
    def load_group(g_):
        io = {}
        gsl = slice(g_ * GN, (g_ + 1) * GN)
        io["cos"] = cspool.tile([P, GN], bf16, tag="cos", name=f"cos{g_}")
        nc.sync.dma_start(io["cos"][:], cosT[:, gsl])
        io["sin"] = cspool.tile([P, GN], bf16, tag="sin", name=f"sin{g_}")
        nc.sync.dma_start(io["sin"][:], sinT[:, gsl])
        io["x"] = []
        for k in range(KT):
            if g_ == 0:
                nc.sync.dma_start(wqk_t[k][:], wqkT[k * P:(k + 1) * P, :])
            x_ = xpool.tile([P, GN], bf16, tag="xt", name=f"xt{g_}_{k}")
            nc.sync.dma_start(x_[:], xT[k * P:(k + 1) * P, gsl])
            io["x"].append(x_)
        io["q"] = [qpool.tile([P, GN], bf16, tag=f"q{p}", name=f"qT{g_}_{p}")
                   for p in range(4)]
        return io

    def qkv_chunk(g_, f, io):
        # one 128-feature tile of the fused q/k projection + RoPE
        gsl = slice(g_ * GN, (g_ + 1) * GN)
        mm_ps = psmm.tile([P, GN], f32, tag="mm", name=f"qkps{g_}_{f}")
        for k in range(KT):
            nc.tensor.matmul(mm_ps[:], wqk_t[k][:, f * P:(f + 1) * P],
                             io["x"][k][:], start=(k == 0), stop=(k == KT - 1))
        raw = rawp.tile([P, GN], bf16, tag="raw", name=f"raw{g_}_{f}")
        # PSUM evacuation + per-partition bias on the scalar engine
        # (keeps DVE free for the RoPE combines).
        nc.scalar.add(raw[:], mm_ps[:], bqk_t[:, f:f + 1])
        rot_ps = psmm.tile([P, GN], f32, tag="mm", name=f"rotps{g_}_{f}")
        nc.tensor.matmul(rot_ps[:], rmat_t[:], raw[:], start=True, stop=True)
        tmp = tmpp.tile([P, GN], bf16, tag="tmp", name=f"tmp{g_}_{f}")
        nc.vector.tensor_mul(tmp[:], rot_ps[:], io["sin"][:])
        dst = io["q"][f][:] if f < 4 else kT_t[f - 4][:, gsl]
        nc.vector.tensor_mul(dst, raw[:], io["cos"][:])
        nc.vector.tensor_add(dst, dst, tmp[:])

    def v_chunk(g_, tt, io):
        ttg = g_ * 4 + tt
        if g_ == 0 and tt == 0:
            for k in range(KT):
                nc.sync.dma_start(wv_t[k][:], wvT[k * P:(k + 1) * P, :])
        v_ps = psmm.tile([P, GN], f32, tag="mm", name=f"vps{g_}_{tt}")
        for k in range(KT):
            nc.tensor.matmul(v_ps[:], io["x"][k][:, tt * P:(tt + 1) * P],
                             wv_t[k][:], start=(k == 0), stop=(k == KT - 1))
        nc.vector.tensor_add(vp4[:, ttg, :, 0:64],
                             v_ps[:].rearrange("p (h e) -> p h e", h=HPC),
                             bvb_t[:].rearrange("p (h e) -> p h e", h=HPC))

    def proj_chunk(g_, tt, yT):
        # partial output projection of group g_, one 128-token stripe
        if g_ == 0 and tt == 0:
            for k in range(4):
                nc.sync.dma_start(wp_t[k][:], wpT[k * P:(k + 1) * P, :])
        for n in range(2):
            o_ps = psmm.tile([P, GN], f32, tag="mm", name=f"ops{g_}_{tt}_{n}")
            for p in range(4):
                nc.tensor.matmul(o_ps[:], yT[p][:, tt * P:(tt + 1) * P],
                                 wp_t[p][:, n * GN:(n + 1) * GN],
                                 start=(p == 0), stop=(p == 3))
            o_sb = outsb.tile([P, GN], f32, tag="osb",
                              name=f"osb{g_}_{tt}_{n}")
            nc.vector.tensor_copy(o_sb[:], o_ps[:])
            nc.sync.dma_start(
                outp[g_ * GN + tt * P: g_ * GN + (tt + 1) * P,
                     n * GN:(n + 1) * GN], o_sb[:])

    def attention(g, io, chunks):
        # chunks: list of thunks of ready PE work, doled out inside the
        # pair loops right after the pipeline-fill QKs.
        njt = 4 * g + 4                      # kv tiles for this q-group
        yT = [ypool.tile([P, GN], bf16, tag=f"yT{p}", name=f"yT{g}_{p}")
              for p in range(4)]
        qT = io["q"]

        def _finish_norm(p_, rcrows):
            # broadcasts + final normalize muls for pair p_; emitted one
            # pair late so the reciprocal/broadcast latency hides behind
            # the next pair's attention.
            bcb = bcp.tile([P, GN], f32, tag="bcb", name=f"bcb{g}_{p_}")
            nc.gpsimd.partition_broadcast(bcb[0:64, :], rcrows[0][:])
            nc.vector.tensor_mul(yT[p_][0:64, :], yT[p_][0:64, :],
                                 bcb[0:64, :])
            bcb2 = bcp.tile([P, GN], f32, tag="bcb", name=f"bcb2{g}_{p_}")
            nc.gpsimd.partition_broadcast(bcb2[0:64, :], rcrows[1][:])
            nc.sync.dma_start(bcb2[64:128, :], bcb2[0:64, :])
            nc.vector.tensor_mul(yT[p_][64:128, :],
                                 yT[p_][64:128, :], bcb2[64:128, :])

        # distribute chunks over the pairs (skip pair 0 at g=0: its x
        # DMAs are still in flight at kernel start)
        p0 = 1 if g == 0 else 0
        sched = {p: [] for p in range(4)}
        for i, ch in enumerate(chunks):
            sched[p0 + i % (4 - p0)].append(ch)

        pending_norm = None
        for p in range(4):
            yps = [psy.tile([65, GN], f32, tag="y", name=f"yps{g}_{p}_{s}")
                   for s in range(2)]
            # software-pipelined: QK/exp for tile j+1 are issued BEFORE
            # the PV of tile j so the in-order PE queue never stalls on
            # exp.
            prev = None

            # diagonal tiles first: their mask latency hides behind the
            # following full tiles, and the pair ends on a short chain.
            jorder = list(range(4 * g, njt)) + list(range(0, 4 * g))

            def _pv(ji_, a2_, c0_):
                # diagonal tiles only touch columns >= c0_: the ji_==0
                # (r==0) tile initializes the full width, later diagonal
                # tiles accumulate their valid suffix only.
                for s in range(2):
                    h = 2 * p + s
                    nc.tensor.matmul(yps[s][:, c0_:GN],
                                     vp4[:, jorder[ji_], h, :],
                                     a2_[:, s * GN + c0_:(s + 1) * GN],
                                     start=(ji_ == 0), stop=(ji_ == njt - 1))

            for ji in range(njt):
                j = jorder[ji]
                r = j - 4 * g                # >=0 on diagonal tiles
                c0 = max(r, 0) * P           # first valid q column
                sc2 = pssc.tile([P, 2 * GN], f32, tag="sc",
                                name=f"sc{g}_{p}_{j}")
                for s in range(2):
                    hb = s * 64
                    nc.tensor.matmul(
                        sc2[:, s * GN + c0:(s + 1) * GN],
                        kT_t[p][hb:hb + 64, j * P:(j + 1) * P],
                        qT[p][hb:hb + 64, c0:GN],
                        start=True, stop=True)
                a2 = attp.tile([P, 2 * GN], bf16, tag="att",
                               name=f"att{g}_{p}_{j}")
                sc2v = sc2[:].rearrange("p (s q) -> p s q", s=2)
                a2v = a2[:].rearrange("p (s q) -> p s q", s=2)
                nc.scalar.activation(a2v[:, :, c0:GN], sc2v[:, :, c0:GN],
                                     AF.Exp, scale=1.0 / math.sqrt(D))
                if r >= 0:
                    nc.vector.tensor_mul(a2[:, c0:c0 + P],
                                         a2[:, c0:c0 + P], dmask_t[:])
                    nc.vector.tensor_mul(a2[:, GN + c0:GN + c0 + P],
                                         a2[:, GN + c0:GN + c0 + P],
                                         dmask_t[:])
                if ji == 1:
                    for ch in sched[p]:
                        ch()
                if prev is not None:
                    _pv(*prev)
                prev = (ji, a2, c0)
            _pv(*prev)
            if pending_norm is not None:
                _finish_norm(*pending_norm)
            # tail: evacuate the y-body fast (frees the yps banks for the
            # next pair) and take reciprocals of the ones-row denominator
            # directly out of PSUM.
            rcrows = []
            for s in range(2):
                hb = s * 64
                nc.vector.tensor_copy(yT[p][hb:hb + 64, :], yps[s][0:64, :])
                rcrow = rcp.tile([1, GN], f32, tag="rcrow",
                                 name=f"rcrow{g}_{p}_{s}")
                nc.vector.reciprocal(rcrow[:], yps[s][64:65, :])
                rcrows.append(rcrow)
            pending_norm = (p, rcrows)
        _finish_norm(*pending_norm)
        return yT

    # ---- main schedule ----
    io_cur = load_group(0)
    for f in range(8):
        qkv_chunk(0, f, io_cur)
    for tt in range(4):
        v_chunk(0, tt, io_cur)

    yT_prev = None
    for g in range(TG):
        io_next = load_group(g + 1) if g + 1 < TG else None
        chunks = []
        if yT_prev is not None:
            chunks += [(lambda tt=tt, y=yT_prev: proj_chunk(g - 1, tt, y))
                       for tt in range(4)]
        if io_next is not None:
            chunks += [(lambda f=f, io=io_next: qkv_chunk(g + 1, f, io))
                       for f in range(8)]
            chunks += [(lambda tt=tt, io=io_next: v_chunk(g + 1, tt, io))
                       for tt in range(4)]
        yT_prev = attention(g, io_cur, chunks)
        io_cur = io_next
    for tt in range(4):
        proj_chunk(TG - 1, tt, yT_prev)


def build_nc():
    nc = bacc.Bacc("TRN2", target_bir_lowering=False, debug=False,
                   num_devices=NCORES)
    xT = nc.dram_tensor("xT", [C, T], bf16, kind="ExternalInput").ap()
    wqkT = nc.dram_tensor("wqkT", [C, 1024], bf16, kind="ExternalInput").ap()
    wvT = nc.dram_tensor("wvT", [C, 512], bf16, kind="ExternalInput").ap()
    wpT = nc.dram_tensor("wpT", [512, 1024], bf16, kind="ExternalInput").ap()
    bqk = nc.dram_tensor("bqk", [P, 8], f32, kind="ExternalInput").ap()
    bvb = nc.dram_tensor("bvb", [P, 512], bf16, kind="ExternalInput").ap()
    cosT = nc.dram_tensor("cosT", [P, T], bf16, kind="ExternalInput").ap()
    sinT = nc.dram_tensor("sinT", [P, T], bf16, kind="ExternalInput").ap()
    rmat = nc.dram_tensor("rmat", [P, P], bf16, kind="ExternalInput").ap()
    dmask = nc.dram_tensor("dmask", [P, P], bf16, kind="ExternalInput").ap()
    outp = nc.dram_tensor("outp", [T, C], f32, kind="ExternalOutput").ap()
    with tile.TileContext(nc) as tc, \
            nc.allow_low_precision(reason="bf16 matmul operands"):
        with ExitStack() as ctx:
            _body(ctx, tc, xT, wqkT, wvT, wpT, bqk, bvb, cosT, sinT, rmat,
                  dmask, outp)
    nc.compile()
    return nc


def _host_inputs(x, w_attn, b_attn, w_proj, cos, sin):
    """Build the 8 per-core input dicts."""
    # rotation matrix: ROT @ q == rotate_half(q) in [d] space
    rot = np.zeros((D, D), np.float32)
    for d_ in range(32):
        rot[d_, d_ + 32] = -1.0
        rot[d_ + 32, d_] = 1.0
    rmat = np.zeros((P, P), np.float32)
    rmat[0:D, 0:D] = rot.T
    rmat[D:P, D:P] = rot.T
    dmask = np.triu(np.ones((P, P), np.float32))
    cosT2 = np.ascontiguousarray(
        np.concatenate([cos[0].T, cos[0].T], axis=0))      # [128, T]
    sinT2 = np.ascontiguousarray(np.concatenate([sin[0].T, sin[0].T], axis=0))

    def bf(a):
        return np.ascontiguousarray(a).astype(BF_NP)

    in_maps = []
    for core in range(NCORES):
        b = core // 2
        hg = core % 2
        h0 = hg * HPC
        qrows = slice(h0 * D, (h0 + HPC) * D)              # 512 rows
        krows = slice(C + h0 * D, C + (h0 + HPC) * D)
        vrows = slice(2 * C + h0 * D, 2 * C + (h0 + HPC) * D)
        wqk = np.concatenate([w_attn[qrows], w_attn[krows]], axis=0)  # [1024, C]
        bqk_np = np.concatenate([b_attn[qrows], b_attn[krows]])       # [1024]
        bvb_np = np.broadcast_to(b_attn[vrows].reshape(1, 512), (P, 512))
        in_maps.append({
            "xT": bf(x[b].T),                                  # [C, T]
            "wqkT": bf(wqk.T),                                 # [C, 1024]
            "wvT": bf(w_attn[vrows].T),                        # [C, 512]
            "wpT": bf(w_proj[:, h0 * D:(h0 + HPC) * D].T),
            "bqk": np.ascontiguousarray(bqk_np.reshape(8, P).T),  # [128, 8]
            "bvb": bf(bvb_np),
            "cosT": bf(cosT2),
            "sinT": bf(sinT2),
            "rmat": bf(rmat),
            "dmask": bf(dmask),
        })
    return in_maps


def kernel(x, w_attn, b_attn, w_proj, b_proj, cos, sin):
    global _NC_CACHE
    x = np.asarray(x, np.float32)
    w_attn = np.asarray(w_attn, np.float32)
    b_attn = np.asarray(b_attn, np.float32)
    w_proj = np.asarray(w_proj, np.float32)
    cos = np.asarray(cos, np.float32)
    sin = np.asarray(sin, np.float32)
    b_proj = np.asarray(b_proj, np.float32)

    if _NC_CACHE is None:
        _NC_CACHE = build_nc()
    nc = _NC_CACHE
    in_maps = _host_inputs(x, w_attn, b_attn, w_proj, cos, sin)
    res = run_bass_kernel_spmd(nc, in_maps, core_ids=list(range(NCORES)))
    parts = [res.results[i]["outp"] for i in range(NCORES)]
    out = np.empty((B, T, C), np.float32)
    for b in range(B):
        out[b] = parts[2 * b] + parts[2 * b + 1] + b_proj
    return out
